# revision 4
# baseline (speedup 1.0000x reference)
"""TGN message-passing + GRU memory update on 8 trn2 NeuronCores.

Sharding (hardcoded): nodes sharded across 8 cores (12500 users + 12500
items each). Host computes winner edge ids per node (index-space only,
from src/dst) and compacts each core's working set; the device gathers
winner cross-memories from a per-core compacted table, streams winner
edge features / own memories / time-encode power basis, and computes the
dense GRU update for its node shard.

Device pipeline (bf16 streams, f32 PSUM accumulation):
  - nodes are host-permuted present-first: columns [0, NPRES) hold nodes
    with a winner edge, [NPRES, PADN) nodes without (x = 0). The present
    path needs no validity masking at all, and r/z gates use host-folded
    (Wi_own + Wh) weights; the missing path is Wh-only (3 matmuls).
  - cross memories are gathered feature-major by dma_gather
    (transpose=True) from a compacted per-core table (unique winner
    sources, int16 indices), 896 indices per instruction (SWDGE
    descriptor-ring bound), 14 instructions per direction.
  - winner e-rows are 1:1 with nodes (an edge is last for at most one
    node per direction), so they stream as a dense feature-major slab,
    like own memories; no gather.
  - time encoding cos(t*bf) enters as a degree-5 polynomial in u=t^2:
    host sends U[6,NPRES] (rows u^m) and vpo = C^T @ WiT_te [6,384].
  - gates: 13 matmuls per 448-node present block (cross/own/e K=128 +
    poly K=6, for r/z/n; Wh_n for the gate-h side). The bh_n bias rides
    a fused (ps_h + bhn) * r scalar_tensor_tensor on DVE.
  - GRU pointwise split across ACT (sigmoid/tanh with per-partition
    biases), DVE, and Pool; outputs written feature-major bf16, host
    inverse-permutes/upcasts.
"""
import numpy as np

N_USER = 100000
N_ITEM = 100000
E = 300000
S = 128
T = 128
DE = 128
M = 2 * S + T + DE  # 512

CORES = 8
NPC = 12500
P = 128
BLK = 448            # nodes per compute block
NPBLK = 28           # present blocks
NMBLK = 2            # missing blocks
NPRES = NPBLK * BLK  # 12544 = 14 gather chunks of 896
PADN = (NPBLK + NMBLK) * BLK  # 13440
NCT = NPRES + 1      # cross table rows (last row stays zero)
GCH = 896            # gather chunk (SWDGE ring bound: <=~992, %128==0)
DPOLY = 6

_CACHE = {}

OUT_NAMES = ("outuT", "outiT")

_GCHUNKS = []
_c0 = 0
while _c0 < NPRES:
    _n = min(GCH, NPRES - _c0)
    _GCHUNKS.append((_c0, _n))
    _c0 += _n


def _build_program(reps=1):
    import concourse.bass as bass
    import concourse.mybir as mybir
    import concourse.tile as tile
    from concourse import bacc

    f32 = mybir.dt.float32
    bf16 = mybir.dt.bfloat16
    i16 = mybir.dt.int16

    nc = bacc.Bacc("TRN2", target_bir_lowering=False, debug=False,
                   enable_asserts=True, num_devices=CORES)

    meta = {}
    for d in ("f", "r"):
        meta[d] = dict(
            ct=nc.dram_tensor(f"ct_{d}", [NCT, S], bf16, kind="ExternalInput"),
            ownT=nc.dram_tensor(f"ownT_{d}", [P, PADN], bf16, kind="ExternalInput"),
            eT=nc.dram_tensor(f"eT_{d}", [P, NPRES], bf16, kind="ExternalInput"),
            U=nc.dram_tensor(f"U_{d}", [DPOLY, NPRES], bf16, kind="ExternalInput"),
            gi=nc.dram_tensor(f"gi_{d}", [P, NPRES // 16], i16, kind="ExternalInput"),
        )
    wic_d = nc.dram_tensor("wic", [P, 3 * S], bf16, kind="ExternalInput")
    wie_d = nc.dram_tensor("wie", [P, 3 * S], bf16, kind="ExternalInput")
    wxo_d = nc.dram_tensor("wxo", [P, 3 * S], bf16, kind="ExternalInput")
    whm_d = nc.dram_tensor("whm", [P, 3 * S], bf16, kind="ExternalInput")
    vpo_d = nc.dram_tensor("vpo", [DPOLY, 3 * S], bf16, kind="ExternalInput")
    br_d = nc.dram_tensor("br", [P, 1], f32, kind="ExternalInput")
    bz_d = nc.dram_tensor("bz", [P, 1], f32, kind="ExternalInput")
    bn_d = nc.dram_tensor("bn", [P, 1], f32, kind="ExternalInput")
    bhn_d = nc.dram_tensor("bhn", [P, 1], f32, kind="ExternalInput")

    outu = nc.dram_tensor("outuT", [P, PADN], bf16, kind="ExternalOutput")
    outi = nc.dram_tensor("outiT", [P, PADN], bf16, kind="ExternalOutput")

    AF = mybir.ActivationFunctionType
    OP = mybir.AluOpType

    with tile.TileContext(nc) as tc:
        with tc.tile_pool(name="const", bufs=1) as cpool, \
             tc.tile_pool(name="gat", bufs=1) as gpool, \
             tc.tile_pool(name="str", bufs=4) as spool, \
             tc.tile_pool(name="blk", bufs=3) as bpool, \
             tc.tile_pool(name="ps", bufs=2, space="PSUM") as psum:

            wic = cpool.tile([P, 3 * S], bf16)
            nc.sync.dma_start(wic[:, :], wic_d.ap())
            wie = cpool.tile([P, 3 * S], bf16)
            nc.sync.dma_start(wie[:, :], wie_d.ap())
            wxo = cpool.tile([P, 3 * S], bf16)
            nc.sync.dma_start(wxo[:, :], wxo_d.ap())
            whm = cpool.tile([P, 3 * S], bf16)
            nc.sync.dma_start(whm[:, :], whm_d.ap())
            vpo = cpool.tile([DPOLY, 3 * S], bf16)
            nc.sync.dma_start(vpo[:, :], vpo_d.ap())
            br = cpool.tile([P, 1], f32)
            nc.sync.dma_start(br[:, :], br_d.ap())
            bz = cpool.tile([P, 1], f32)
            nc.sync.dma_start(bz[:, :], bz_d.ap())
            bn = cpool.tile([P, 1], f32)
            nc.sync.dma_start(bn[:, :], bn_d.ap())
            bhn = cpool.tile([P, 1], f32)
            nc.sync.dma_start(bhn[:, :], bhn_d.ap())

            from contextlib import nullcontext
            loop_ctx = tc.For_i(0, reps, 1) if reps > 1 else nullcontext()

            def pointwise(own, bs, ps_r, ps_z, ps_n, ps_h, out_d, j0):
                r = bpool.tile([P, BLK], f32, tag="r")
                nc.scalar.activation(r[:, :], ps_r[:, :], AF.Sigmoid,
                                     bias=br[:, :1])
                z = bpool.tile([P, BLK], bf16, tag="z")
                nc.scalar.activation(z[:, :], ps_z[:, :], AF.Sigmoid,
                                     bias=bz[:, :1])
                t1 = bpool.tile([P, BLK], f32, tag="t1")
                nc.vector.scalar_tensor_tensor(
                    out=t1[:, :], in0=ps_h[:, :], scalar=bhn[:, :1],
                    in1=r[:, :], op0=OP.add, op1=OP.mult)
                if ps_n is not None:
                    nc.vector.tensor_tensor(out=t1[:, :], in0=t1[:, :],
                                            in1=ps_n[:, :], op=OP.add)
                n = bpool.tile([P, BLK], bf16, tag="n")
                nc.scalar.activation(n[:, :], t1[:, :], AF.Tanh,
                                     bias=bn[:, :1])
                dd = bpool.tile([P, BLK], bf16, tag="dd")
                nc.vector.tensor_tensor(out=dd[:, :], in0=own[:, bs],
                                        in1=n[:, :], op=OP.subtract)
                zd = bpool.tile([P, BLK], bf16, tag="zd")
                nc.vector.tensor_tensor(out=zd[:, :], in0=z[:, :],
                                        in1=dd[:, :], op=OP.mult)
                oc = bpool.tile([P, BLK], bf16, tag="oc")
                nc.vector.tensor_tensor(out=oc[:, :], in0=n[:, :],
                                        in1=zd[:, :], op=OP.add)
                nc.sync.dma_start(out_d.ap()[:, j0:j0 + BLK], oc[:, :])

            with loop_ctx:
              for d, out_d in (("f", outi), ("r", outu)):
                md = meta[d]
                gidx = gpool.tile([P, NPRES // 16], i16, tag=f"gidx{d}")
                nc.sync.dma_start(gidx[:, :], md["gi"].ap())

                cgs = []
                for ci, (c0, ncol) in enumerate(_GCHUNKS):
                    cg = gpool.tile([P, 1, GCH], bf16, tag=f"cg{d}{ci}")
                    nc.gpsimd.dma_gather(
                        cg[:, :, :ncol], md["ct"].ap(),
                        gidx[:, c0 // 16:(c0 + ncol) // 16],
                        num_idxs=ncol, num_idxs_reg=ncol,
                        elem_size=S, transpose=True)
                    cgs.append(cg)

                # present blocks: 2 per gather chunk (last chunk: 1)
                for b in range(NPBLK):
                    j0 = b * BLK
                    cg = cgs[b // 2]
                    cs = slice((b % 2) * BLK, (b % 2) * BLK + BLK)
                    own = spool.tile([P, BLK], bf16, tag="own")
                    nc.sync.dma_start(own[:, :],
                                      md["ownT"].ap()[:, j0:j0 + BLK])
                    eg = spool.tile([P, BLK], bf16, tag="eg")
                    nc.sync.dma_start(eg[:, :], md["eT"].ap()[:, j0:j0 + BLK])
                    uu = spool.tile([DPOLY, BLK], bf16, tag="uu")
                    nc.sync.dma_start(uu[:, :], md["U"].ap()[:, j0:j0 + BLK])

                    ps_r = psum.tile([P, BLK], f32, space="PSUM", tag="ps_r")
                    ps_z = psum.tile([P, BLK], f32, space="PSUM", tag="ps_z")
                    ps_n = psum.tile([P, BLK], f32, space="PSUM", tag="ps_n")
                    ps_h = psum.tile([P, BLK], f32, space="PSUM", tag="ps_h")
                    for g, ps in enumerate((ps_r, ps_z, ps_n)):
                        gs = slice(g * S, (g + 1) * S)
                        nc.tensor.matmul(ps[:, :], wic[:, gs], cg[:, 0, cs],
                                         start=True, stop=False)
                        nc.tensor.matmul(ps[:, :], wxo[:, gs], own[:, :],
                                         start=False, stop=False)
                        nc.tensor.matmul(ps[:, :], wie[:, gs], eg[:, :],
                                         start=False, stop=False)
                        nc.tensor.matmul(ps[:, :], vpo[:, gs], uu[:, :],
                                         start=False, stop=True)
                    nc.tensor.matmul(ps_h[:, :], whm[:, 2 * S:3 * S],
                                     own[:, :], start=True, stop=True)
                    pointwise(own, slice(0, BLK), ps_r, ps_z, ps_n, ps_h,
                              out_d, j0)

                # missing blocks: x = 0 -> Wh-only gates
                for b in range(NPBLK, NPBLK + NMBLK):
                    j0 = b * BLK
                    own = spool.tile([P, BLK], bf16, tag="own")
                    nc.sync.dma_start(own[:, :],
                                      md["ownT"].ap()[:, j0:j0 + BLK])
                    ps_r = psum.tile([P, BLK], f32, space="PSUM", tag="ps_r")
                    ps_z = psum.tile([P, BLK], f32, space="PSUM", tag="ps_z")
                    ps_h = psum.tile([P, BLK], f32, space="PSUM", tag="ps_h")
                    for g, ps in enumerate((ps_r, ps_z, ps_h)):
                        gs = slice(g * S, (g + 1) * S)
                        nc.tensor.matmul(ps[:, :], whm[:, gs], own[:, :],
                                         start=True, stop=True)
                    pointwise(own, slice(0, BLK), ps_r, ps_z, None, ps_h,
                              out_d, j0)

    nc.compile()
    return nc


def _np_gru(x, h, Wi, Wh, bi, bh):
    gi = x @ Wi.T + bi
    gh = h @ Wh.T + bh
    gir, giz, gin = np.split(gi, 3, axis=1)
    ghr, ghz, ghn = np.split(gh, 3, axis=1)
    r = 1.0 / (1.0 + np.exp(-(gir + ghr)))
    z = 1.0 / (1.0 + np.exp(-(giz + ghz)))
    n = np.tanh(gin + r * ghn)
    return (1.0 - z) * n + z * h


def _host_prep(si, sj, t, e, src, dst, Wi, Wh, bi, bh, basis_freq):
    import ml_dtypes
    bf16 = ml_dtypes.bfloat16

    eid = np.arange(E, dtype=np.int64)
    lastf = np.full(N_ITEM, -1, dtype=np.int64)
    lastf[dst.astype(np.int64)] = eid
    lastr = np.full(N_USER, -1, dtype=np.int64)
    lastr[src.astype(np.int64)] = eid

    # te poly: cos(t*f) = sum_m C[k,m] * (t^2)^m,  C[k,m] = (-1)^m f^(2m)/(2m)!
    import math
    bf = np.asarray(basis_freq, np.float64)
    fact = np.array([math.factorial(2 * m) for m in range(DPOLY)], np.float64)
    C = np.stack([((-1.0) ** m) * bf ** (2 * m) / fact[m]
                  for m in range(DPOLY)], axis=1)  # [T, DPOLY]
    WiT = np.ascontiguousarray(Wi.T).astype(np.float64)
    WhT = np.ascontiguousarray(Wh.T).astype(np.float64)
    vpo = (C.T @ WiT[2 * S:2 * S + T]).astype(np.float32)  # [DPOLY, 384]

    wic = WiT[0:S].astype(bf16)
    wie = WiT[2 * S + T:].astype(bf16)
    wxo = WiT[S:2 * S].copy()
    wxo[:, :2 * S] += WhT[:, :2 * S]  # fold Wh into Wi_own for r,z gates
    wxo = wxo.astype(bf16)
    whm = WhT.astype(bf16)
    br = (bi[:S] + bh[:S]).reshape(P, 1).astype(np.float32)
    bz = (bi[S:2 * S] + bh[S:2 * S]).reshape(P, 1).astype(np.float32)
    bn = bi[2 * S:].reshape(P, 1).astype(np.float32)
    bhn = bh[2 * S:].reshape(P, 1).astype(np.float32)

    t64 = np.asarray(t, np.float64)
    patches = []   # (side 'u'/'i', node_global_ids, x_rows, h_rows)
    in_maps = []
    colmaps = []

    def meta_for(w, own_rows, cross_idx_all, cross_mem, t64, e):
        npc = len(w)
        present = np.nonzero(w >= 0)[0]
        missing = np.nonzero(w < 0)[0]
        over_p = present[NPRES:] if len(present) > NPRES else np.empty(0, np.int64)
        over_m = missing[NMBLK * BLK:] if len(missing) > NMBLK * BLK else np.empty(0, np.int64)
        present = present[:NPRES]
        missing = missing[:NMBLK * BLK]
        np_, nm_ = len(present), len(missing)

        col_of = np.full(npc, PADN - 1, np.int64)
        col_of[present] = np.arange(np_)
        col_of[missing] = NPRES + np.arange(nm_)

        wp = w[present]
        u = t64[wp] ** 2
        U = np.zeros((DPOLY, NPRES), np.float32)
        for m in range(DPOLY):
            U[m, :np_] = u ** m
        eT = np.zeros((P, NPRES), np.float32)
        eT[:, :np_] = e[wp].T
        ownT = np.zeros((P, PADN), np.float32)
        ownT[:, :np_] = own_rows[present].T
        ownT[:, NPRES:NPRES + nm_] = own_rows[missing].T

        cids = cross_idx_all[wp].astype(np.int64)
        uniq, inv = np.unique(cids, return_inverse=True)
        ct = np.zeros((NCT, S), np.float32)
        ct[:len(uniq)] = cross_mem[uniq]
        idx = np.full(NPRES, NCT - 1, np.int32)
        idx[:np_] = inv
        idx16 = idx.astype(np.int16)
        gi = np.tile(idx16.reshape(-1, 16).T, (8, 1)).copy()

        m = dict(ct=ct.astype(bf16), ownT=ownT.astype(bf16),
                 eT=eT.astype(bf16), U=U.astype(bf16), gi=gi)
        return m, col_of, over_p, over_m, wp

    for c in range(CORES):
        sl = slice(c * NPC, (c + 1) * NPC)
        im = {"wic": wic, "wie": wie, "wxo": wxo, "whm": whm,
              "vpo": vpo.astype(bf16), "br": br, "bz": bz, "bn": bn,
              "bhn": bhn}
        cm = {}
        for d, w_all, own_rows, cross_idx_all, cross_mem, side in (
                ("f", lastf[sl], sj[sl], src, si, "i"),
                ("r", lastr[sl], si[sl], dst, sj, "u")):
            m, col_of, over_p, over_m, wp = meta_for(
                w_all, own_rows, cross_idx_all, cross_mem, t64, e)
            for k, v in m.items():
                im[f"{k}_{d}"] = v
            cm[d] = col_of
            for over, is_present in ((over_p, True), (over_m, False)):
                if len(over):
                    h_rows = own_rows[over]
                    if is_present:
                        wo = w_all[over]
                        te = np.cos(np.asarray(t, np.float32)[wo][:, None]
                                    * np.asarray(basis_freq, np.float32)[None, :])
                        x = np.concatenate(
                            [cross_mem[cross_idx_all[wo].astype(np.int64)],
                             h_rows, te, e[wo]], axis=1)
                    else:
                        x = np.zeros((len(over), M), np.float32)
                    out = _np_gru(x, h_rows, Wi, Wh, bi, bh)
                    patches.append((c, side, over, out))
        in_maps.append(im)
        colmaps.append(cm)
    return in_maps, colmaps, patches


def _postprocess(results, colmaps, patches):
    si_new = np.empty((N_USER, S), np.float32)
    sj_new = np.empty((N_ITEM, S), np.float32)
    for c in range(CORES):
        sl = slice(c * NPC, (c + 1) * NPC)
        outu = np.asarray(results[c]["outuT"]).T.astype(np.float32)
        outi = np.asarray(results[c]["outiT"]).T.astype(np.float32)
        si_new[sl] = outu[colmaps[c]["r"]]
        sj_new[sl] = outi[colmaps[c]["f"]]
    for c, side, nodes, out in patches:
        if side == "u":
            si_new[c * NPC + nodes] = out
        else:
            sj_new[c * NPC + nodes] = out
    return si_new, sj_new


def kernel(si, sj, t, e, src, dst, Wi, Wh, bi, bh, basis_freq):
    from concourse import bass_utils

    si = np.asarray(si, np.float32)
    sj = np.asarray(sj, np.float32)
    t = np.asarray(t, np.float32)
    e = np.asarray(e, np.float32)
    src = np.asarray(src, np.int32)
    dst = np.asarray(dst, np.int32)
    Wi = np.asarray(Wi, np.float32)
    Wh = np.asarray(Wh, np.float32)
    bi = np.asarray(bi, np.float32)
    bh = np.asarray(bh, np.float32)
    basis_freq = np.asarray(basis_freq, np.float32)

    if "nc" not in _CACHE:
        _CACHE["nc"] = _build_program()
    nc = _CACHE["nc"]

    in_maps, colmaps, patches = _host_prep(si, sj, t, e, src, dst,
                                           Wi, Wh, bi, bh, basis_freq)
    res = bass_utils.run_bass_kernel_spmd(nc, in_maps, core_ids=list(range(CORES)))
    return _postprocess(res.results, colmaps, patches)


import concourse.bass as bass  # noqa: E402
import concourse.mybir as mybir  # noqa: E402


# revision 25
# speedup vs baseline: 2.3150x; 2.3150x over previous
"""TGN message-passing + GRU memory update on 8 trn2 NeuronCores.

Sharding (hardcoded): nodes sharded across 8 cores (12500 users + 12500
items each). Host computes winner edge ids per node (index-space only,
from src/dst) and compacts each core's working set; the device gathers
winner cross-memories from a per-core compacted table, streams winner
edge features / own memories / time-encode power basis, and computes the
dense GRU update for its node shard.

Device pipeline (bf16 streams, f32 PSUM accumulation):
  - nodes are host-permuted present-first: columns [0, NPRES) hold nodes
    with a winner edge, [NPRES, PADN) nodes without (x = 0). The present
    path needs no validity masking at all, and r/z gates use host-folded
    (Wi_own + Wh) weights; the missing path is Wh-only (3 matmuls).
  - winner cross-memories / e-rows are host-compacted into dense
    feature-major per-core slabs in node order (an edge is last for at
    most one node per direction) and streamed like the own memories:
    one big HWDGE DMA per slab per direction, no per-row transfers.
    (A device-side dma_gather(transpose=True) path from a compacted
    int16-indexed table is kept behind pre_cross=False; it measures
    ~5-10% slower end-to-end due to SWDGE descriptor-generation cost.)
  - time encoding cos(t*bf) enters as a degree-5 polynomial in u=t^2:
    host sends U[6,NPRES] (rows u^m) and vpo = C^T @ WiT_te [6,384].
  - gates: 13 matmuls per 448-node present block (cross/own/e K=128 +
    poly K=6, for r/z/n; Wh_n for the gate-h side). The bh_n bias rides
    a fused (ps_h + bhn) * r scalar_tensor_tensor on DVE.
  - pointwise is software-pipelined in two stages so PSUM banks free
    with minimal PE stalling: stage1 (sigmoids + (ps_h+bhn)*r + ps_n
    add) per block, stage2 (tanh + GRU combine + writeback) deferred
    behind the next block's matmuls. Outputs written feature-major
    bf16; host inverse-permutes/upcasts.
"""
import numpy as np

N_USER = 100000
N_ITEM = 100000
E = 300000
S = 128
T = 128
DE = 128
M = 2 * S + T + DE  # 512

CORES = 8
NPC = 12500
P = 128
BLK = 448            # nodes per compute block
NPBLK = 28           # present blocks
NMBLK = 2            # missing blocks
NPRES = NPBLK * BLK  # 12544 = 14 gather chunks of 896
PADN = (NPBLK + NMBLK) * BLK  # 13440
NCT = NPRES + 1      # cross table rows (last row stays zero)
GCH = 896            # gather chunk (SWDGE ring bound: <=~992, %128==0)
DPOLY = 6

_CACHE = {}

OUT_NAMES = ("outuT", "outiT")

_GCHUNKS = []
_c0 = 0
while _c0 < NPRES:
    _n = min(GCH, NPRES - _c0)
    _GCHUNKS.append((_c0, _n))
    _c0 += _n


def _build_program(reps=1, no_gather=False, only_gather=False, gq=4,
                   pre_cross=True, mm_only=False):
    import concourse.bass as bass
    import concourse.mybir as mybir
    import concourse.tile as tile
    from concourse import bacc

    f32 = mybir.dt.float32
    bf16 = mybir.dt.bfloat16
    i16 = mybir.dt.int16

    nc = bacc.Bacc("TRN2", target_bir_lowering=False, debug=False,
                   enable_asserts=True, num_devices=CORES,
                   num_swdge_queues=gq)

    meta = {}
    for d in ("f", "r"):
        meta[d] = dict(
            ownT=nc.dram_tensor(f"ownT_{d}", [P, PADN], bf16, kind="ExternalInput"),
            eT=nc.dram_tensor(f"eT_{d}", [P, NPRES], bf16, kind="ExternalInput"),
            U=nc.dram_tensor(f"U_{d}", [DPOLY, NPRES], bf16, kind="ExternalInput"),
        )
        if pre_cross:
            meta[d]["cT"] = nc.dram_tensor(f"cT_{d}", [P, NPRES], bf16,
                                           kind="ExternalInput")
        else:
            meta[d]["ct"] = nc.dram_tensor(f"ct_{d}", [NCT, S], bf16,
                                           kind="ExternalInput")
            meta[d]["gi"] = nc.dram_tensor(f"gi_{d}", [P, NPRES // 16], i16,
                                           kind="ExternalInput")
    wic_d = nc.dram_tensor("wic", [P, 3 * S], bf16, kind="ExternalInput")
    wie_d = nc.dram_tensor("wie", [P, 3 * S], bf16, kind="ExternalInput")
    wxo_d = nc.dram_tensor("wxo", [P, 3 * S], bf16, kind="ExternalInput")
    whm_d = nc.dram_tensor("whm", [P, 3 * S], bf16, kind="ExternalInput")
    vpo_d = nc.dram_tensor("vpo", [DPOLY, 3 * S], bf16, kind="ExternalInput")
    br_d = nc.dram_tensor("br", [P, 1], f32, kind="ExternalInput")
    bz_d = nc.dram_tensor("bz", [P, 1], f32, kind="ExternalInput")
    bn_d = nc.dram_tensor("bn", [P, 1], f32, kind="ExternalInput")
    bhn_d = nc.dram_tensor("bhn", [P, 1], f32, kind="ExternalInput")

    outu = nc.dram_tensor("outuT", [P, PADN], bf16, kind="ExternalOutput")
    outi = nc.dram_tensor("outiT", [P, PADN], bf16, kind="ExternalOutput")

    AF = mybir.ActivationFunctionType
    OP = mybir.AluOpType

    with tile.TileContext(nc) as tc:
        with tc.tile_pool(name="const", bufs=1) as cpool, \
             tc.tile_pool(name="gat", bufs=1) as gpool, \
             tc.tile_pool(name="str", bufs=2) as spool, \
             tc.tile_pool(name="blk", bufs=2) as bpool, \
             tc.tile_pool(name="ps", bufs=2, space="PSUM") as psum:

            wic = cpool.tile([P, 3 * S], bf16)
            nc.sync.dma_start(wic[:, :], wic_d.ap())
            wie = cpool.tile([P, 3 * S], bf16)
            nc.sync.dma_start(wie[:, :], wie_d.ap())
            wxo = cpool.tile([P, 3 * S], bf16)
            nc.sync.dma_start(wxo[:, :], wxo_d.ap())
            whm = cpool.tile([P, 3 * S], bf16)
            nc.sync.dma_start(whm[:, :], whm_d.ap())
            vpo = cpool.tile([DPOLY, 3 * S], bf16)
            nc.sync.dma_start(vpo[:, :], vpo_d.ap())
            br = cpool.tile([P, 1], f32)
            nc.sync.dma_start(br[:, :], br_d.ap())
            bz = cpool.tile([P, 1], f32)
            nc.sync.dma_start(bz[:, :], bz_d.ap())
            bn = cpool.tile([P, 1], f32)
            nc.sync.dma_start(bn[:, :], bn_d.ap())
            bhn = cpool.tile([P, 1], f32)
            nc.sync.dma_start(bhn[:, :], bhn_d.ap())

            from contextlib import nullcontext
            loop_ctx = tc.For_i(0, reps, 1) if reps > 1 else nullcontext()

            def stage1(ps_r, ps_z, ps_n, ps_h):
                """Release all PSUM banks ASAP: sigmoids + (ps_h+bhn)*r + ps_n."""
                if mm_only:
                    return None
                r = bpool.tile([P, BLK], f32, tag="r", bufs=3)
                nc.scalar.activation(r[:, :], ps_r[:, :], AF.Sigmoid,
                                     bias=br[:, :1])
                z = bpool.tile([P, BLK], bf16, tag="z", bufs=3)
                nc.scalar.activation(z[:, :], ps_z[:, :], AF.Sigmoid,
                                     bias=bz[:, :1])
                t1 = bpool.tile([P, BLK], f32, tag="t1", bufs=3)
                nc.vector.scalar_tensor_tensor(
                    out=t1[:, :], in0=ps_h[:, :], scalar=bhn[:, :1],
                    in1=r[:, :], op0=OP.add, op1=OP.mult)
                if ps_n is not None:
                    nc.vector.tensor_tensor(out=t1[:, :], in0=t1[:, :],
                                            in1=ps_n[:, :], op=OP.add)
                return (z, t1)

            def stage2(st, own, bs, out_d, j0):
                """Deferred tail: tanh + GRU combine + writeback."""
                if st is None:
                    return
                z, t1 = st
                n = bpool.tile([P, BLK], bf16, tag="n", bufs=3)
                nc.scalar.activation(n[:, :], t1[:, :], AF.Tanh,
                                     bias=bn[:, :1])
                dd = bpool.tile([P, BLK], bf16, tag="dd", bufs=3)
                nc.vector.tensor_tensor(out=dd[:, :], in0=own[:, bs],
                                        in1=n[:, :], op=OP.subtract)
                zd = bpool.tile([P, BLK], bf16, tag="zd", bufs=3)
                nc.vector.tensor_tensor(out=zd[:, :], in0=z[:, :],
                                        in1=dd[:, :], op=OP.mult)
                oc = bpool.tile([P, BLK], bf16, tag="oc", bufs=3)
                nc.vector.tensor_tensor(out=oc[:, :], in0=n[:, :],
                                        in1=zd[:, :], op=OP.add)
                nc.scalar.dma_start(out_d.ap()[:, j0:j0 + BLK], oc[:, :])

            UCH = 4 * BLK  # U streamed in 4-block chunks

            with loop_ctx:
              # prefetch: both directions' slabs + idx/table streams up front
              slabs = {}
              for d in ("f", "r"):
                md = meta[d]
                if pre_cross:
                    cgb = gpool.tile([P, NPRES], bf16, tag=f"cgb{d}")
                    nc.scalar.dma_start(cgb[:, :], md["cT"].ap())
                    slabs[d] = dict(cgb=cgb)
                else:
                    gidx = gpool.tile([P, NPRES // 16], i16, tag=f"gidx{d}")
                    nc.scalar.dma_start(gidx[:, :], md["gi"].ap())
                    slabs[d] = dict(gidx=gidx)
                if not only_gather:
                    ownb = gpool.tile([P, PADN], bf16, tag=f"own{d}")
                    nc.sync.dma_start(ownb[:, :], md["ownT"].ap())
                    egb = gpool.tile([P, NPRES], bf16, tag=f"eg{d}")
                    nc.sync.dma_start(egb[:, :], md["eT"].ap())
                    slabs[d]["ownb"] = ownb
                    slabs[d]["egb"] = egb

              for d, out_d in (("f", outi), ("r", outu)):
                md = meta[d]
                if pre_cross:
                    cgb = slabs[d]["cgb"]
                    cg_of = lambda b: cgb[:, b * BLK:(b + 1) * BLK]
                else:
                    gidx = slabs[d]["gidx"]
                    cgs = []
                    for ci, (c0, ncol) in enumerate(_GCHUNKS):
                        cg = gpool.tile([P, 1, GCH], bf16, tag=f"cg{d}{ci}")
                        if not no_gather:
                            nc.gpsimd.dma_gather(
                                cg[:, :, :ncol], md["ct"].ap(),
                                gidx[:, c0 // 16:(c0 + ncol) // 16],
                                num_idxs=ncol, num_idxs_reg=ncol,
                                elem_size=S, transpose=True,
                                queue_num=ci % gq)
                        cgs.append(cg)
                    cg_of = lambda b: cgs[b // 2][:, 0,
                                                  (b % 2) * BLK:(b % 2 + 1) * BLK]
                if only_gather:
                    continue
                ownb = slabs[d]["ownb"]
                egb = slabs[d]["egb"]

                # blocks, software-pipelined: stage2(b-1) rides behind
                # stage1(b) so PSUM banks free with minimal PE stalling
                uu = None
                pending = None
                for b in range(NPBLK + NMBLK):
                    j0 = b * BLK
                    present = b < NPBLK
                    if present and b % 4 == 0:
                        uu = spool.tile([DPOLY, UCH], bf16, tag="uu")
                        nc.scalar.dma_start(uu[:, :],
                                            md["U"].ap()[:, j0:j0 + UCH])
                    us = slice((b % 4) * BLK, (b % 4 + 1) * BLK)

                    ps_r = psum.tile([P, BLK], f32, space="PSUM", tag="ps_r")
                    ps_z = psum.tile([P, BLK], f32, space="PSUM", tag="ps_z")
                    ps_h = psum.tile([P, BLK], f32, space="PSUM", tag="ps_h")
                    if present:
                        ps_n = psum.tile([P, BLK], f32, space="PSUM", tag="ps_n")
                        cgv = cg_of(b)
                        for g, ps in enumerate((ps_r, ps_z, ps_n)):
                            gs = slice(g * S, (g + 1) * S)
                            nc.tensor.matmul(ps[:, :], wic[:, gs], cgv,
                                             start=True, stop=False)
                            nc.tensor.matmul(ps[:, :], wxo[:, gs],
                                             ownb[:, j0:j0 + BLK],
                                             start=False, stop=False)
                            nc.tensor.matmul(ps[:, :], wie[:, gs],
                                             egb[:, j0:j0 + BLK],
                                             start=False, stop=False)
                            nc.tensor.matmul(ps[:, :], vpo[:, gs], uu[:, us],
                                             start=False, stop=True)
                        nc.tensor.matmul(ps_h[:, :], whm[:, 2 * S:3 * S],
                                         ownb[:, j0:j0 + BLK],
                                         start=True, stop=True)
                    else:
                        ps_n = None
                        for g, ps in enumerate((ps_r, ps_z, ps_h)):
                            gs = slice(g * S, (g + 1) * S)
                            nc.tensor.matmul(ps[:, :], whm[:, gs],
                                             ownb[:, j0:j0 + BLK],
                                             start=True, stop=True)
                    st = stage1(ps_r, ps_z, ps_n, ps_h)
                    if pending is not None:
                        stage2(*pending)
                    pending = (st, ownb, slice(j0, j0 + BLK), out_d, j0)
                if pending is not None:
                    stage2(*pending)

    nc.compile()
    return nc


def _np_gru(x, h, Wi, Wh, bi, bh):
    gi = x @ Wi.T + bi
    gh = h @ Wh.T + bh
    gir, giz, gin = np.split(gi, 3, axis=1)
    ghr, ghz, ghn = np.split(gh, 3, axis=1)
    r = 1.0 / (1.0 + np.exp(-(gir + ghr)))
    z = 1.0 / (1.0 + np.exp(-(giz + ghz)))
    n = np.tanh(gin + r * ghn)
    return (1.0 - z) * n + z * h


def _host_prep(si, sj, t, e, src, dst, Wi, Wh, bi, bh, basis_freq):
    import ml_dtypes
    bf16 = ml_dtypes.bfloat16

    eid = np.arange(E, dtype=np.int64)
    lastf = np.full(N_ITEM, -1, dtype=np.int64)
    lastf[dst.astype(np.int64)] = eid
    lastr = np.full(N_USER, -1, dtype=np.int64)
    lastr[src.astype(np.int64)] = eid

    # te poly: cos(t*f) = sum_m C[k,m] * (t^2)^m,  C[k,m] = (-1)^m f^(2m)/(2m)!
    import math
    bf = np.asarray(basis_freq, np.float64)
    fact = np.array([math.factorial(2 * m) for m in range(DPOLY)], np.float64)
    C = np.stack([((-1.0) ** m) * bf ** (2 * m) / fact[m]
                  for m in range(DPOLY)], axis=1)  # [T, DPOLY]
    WiT = np.ascontiguousarray(Wi.T).astype(np.float64)
    WhT = np.ascontiguousarray(Wh.T).astype(np.float64)
    vpo = (C.T @ WiT[2 * S:2 * S + T]).astype(np.float32)  # [DPOLY, 384]

    wic = WiT[0:S].astype(bf16)
    wie = WiT[2 * S + T:].astype(bf16)
    wxo = WiT[S:2 * S].copy()
    wxo[:, :2 * S] += WhT[:, :2 * S]  # fold Wh into Wi_own for r,z gates
    wxo = wxo.astype(bf16)
    whm = WhT.astype(bf16)
    br = (bi[:S] + bh[:S]).reshape(P, 1).astype(np.float32)
    bz = (bi[S:2 * S] + bh[S:2 * S]).reshape(P, 1).astype(np.float32)
    bn = bi[2 * S:].reshape(P, 1).astype(np.float32)
    bhn = bh[2 * S:].reshape(P, 1).astype(np.float32)

    t64 = np.asarray(t, np.float64)
    patches = []   # (side 'u'/'i', node_global_ids, x_rows, h_rows)
    in_maps = []
    colmaps = []

    def meta_for(w, own_rows, cross_idx_all, cross_mem, t64, e):
        npc = len(w)
        present = np.nonzero(w >= 0)[0]
        missing = np.nonzero(w < 0)[0]
        over_p = present[NPRES:] if len(present) > NPRES else np.empty(0, np.int64)
        over_m = missing[NMBLK * BLK:] if len(missing) > NMBLK * BLK else np.empty(0, np.int64)
        present = present[:NPRES]
        missing = missing[:NMBLK * BLK]
        np_, nm_ = len(present), len(missing)

        col_of = np.full(npc, PADN - 1, np.int64)
        col_of[present] = np.arange(np_)
        col_of[missing] = NPRES + np.arange(nm_)

        wp = w[present]
        u = t64[wp] ** 2
        U = np.zeros((DPOLY, NPRES), np.float32)
        for m in range(DPOLY):
            U[m, :np_] = u ** m
        eT = np.zeros((P, NPRES), np.float32)
        eT[:, :np_] = e[wp].T
        ownT = np.zeros((P, PADN), np.float32)
        ownT[:, :np_] = own_rows[present].T
        ownT[:, NPRES:NPRES + nm_] = own_rows[missing].T

        cids = cross_idx_all[wp].astype(np.int64)
        uniq, inv = np.unique(cids, return_inverse=True)
        ct = np.zeros((NCT, S), np.float32)
        ct[:len(uniq)] = cross_mem[uniq]
        idx = np.full(NPRES, NCT - 1, np.int32)
        idx[:np_] = inv
        idx16 = idx.astype(np.int16)
        gi = np.tile(idx16.reshape(-1, 16).T, (8, 1)).copy()
        cT = np.zeros((P, NPRES), np.float32)
        cT[:, :np_] = cross_mem[cids].T

        m = dict(ct=ct.astype(bf16), ownT=ownT.astype(bf16),
                 eT=eT.astype(bf16), U=U.astype(bf16), gi=gi,
                 cT=cT.astype(bf16))
        return m, col_of, over_p, over_m, wp

    for c in range(CORES):
        sl = slice(c * NPC, (c + 1) * NPC)
        im = {"wic": wic, "wie": wie, "wxo": wxo, "whm": whm,
              "vpo": vpo.astype(bf16), "br": br, "bz": bz, "bn": bn,
              "bhn": bhn}
        cm = {}
        for d, w_all, own_rows, cross_idx_all, cross_mem, side in (
                ("f", lastf[sl], sj[sl], src, si, "i"),
                ("r", lastr[sl], si[sl], dst, sj, "u")):
            m, col_of, over_p, over_m, wp = meta_for(
                w_all, own_rows, cross_idx_all, cross_mem, t64, e)
            for k, v in m.items():
                im[f"{k}_{d}"] = v
            cm[d] = col_of
            for over, is_present in ((over_p, True), (over_m, False)):
                if len(over):
                    h_rows = own_rows[over]
                    if is_present:
                        wo = w_all[over]
                        te = np.cos(np.asarray(t, np.float32)[wo][:, None]
                                    * np.asarray(basis_freq, np.float32)[None, :])
                        x = np.concatenate(
                            [cross_mem[cross_idx_all[wo].astype(np.int64)],
                             h_rows, te, e[wo]], axis=1)
                    else:
                        x = np.zeros((len(over), M), np.float32)
                    out = _np_gru(x, h_rows, Wi, Wh, bi, bh)
                    patches.append((c, side, over, out))
        in_maps.append(im)
        colmaps.append(cm)
    return in_maps, colmaps, patches


def _postprocess(results, colmaps, patches):
    si_new = np.empty((N_USER, S), np.float32)
    sj_new = np.empty((N_ITEM, S), np.float32)
    for c in range(CORES):
        sl = slice(c * NPC, (c + 1) * NPC)
        outu = np.asarray(results[c]["outuT"]).T.astype(np.float32)
        outi = np.asarray(results[c]["outiT"]).T.astype(np.float32)
        si_new[sl] = outu[colmaps[c]["r"]]
        sj_new[sl] = outi[colmaps[c]["f"]]
    for c, side, nodes, out in patches:
        if side == "u":
            si_new[c * NPC + nodes] = out
        else:
            sj_new[c * NPC + nodes] = out
    return si_new, sj_new


def kernel(si, sj, t, e, src, dst, Wi, Wh, bi, bh, basis_freq):
    from concourse import bass_utils

    si = np.asarray(si, np.float32)
    sj = np.asarray(sj, np.float32)
    t = np.asarray(t, np.float32)
    e = np.asarray(e, np.float32)
    src = np.asarray(src, np.int32)
    dst = np.asarray(dst, np.int32)
    Wi = np.asarray(Wi, np.float32)
    Wh = np.asarray(Wh, np.float32)
    bi = np.asarray(bi, np.float32)
    bh = np.asarray(bh, np.float32)
    basis_freq = np.asarray(basis_freq, np.float32)

    if "nc" not in _CACHE:
        _CACHE["nc"] = _build_program()
    nc = _CACHE["nc"]

    in_maps, colmaps, patches = _host_prep(si, sj, t, e, src, dst,
                                           Wi, Wh, bi, bh, basis_freq)
    res = bass_utils.run_bass_kernel_spmd(nc, in_maps, core_ids=list(range(CORES)))
    return _postprocess(res.results, colmaps, patches)


import concourse.bass as bass  # noqa: E402
import concourse.mybir as mybir  # noqa: E402


# revision 29
# speedup vs baseline: 2.7456x; 1.1860x over previous
"""TGN message-passing + GRU memory update on 8 trn2 NeuronCores.

Sharding (hardcoded): nodes sharded across 8 cores (12500 users + 12500
items each). Host computes winner edge ids per node (index-space only,
from src/dst) and compacts each core's working set; the device gathers
winner cross-memories from a per-core compacted table, streams winner
edge features / own memories / time-encode power basis, and computes the
dense GRU update for its node shard.

Device pipeline (bf16 streams, f32 PSUM accumulation):
  - nodes are host-permuted present-first: columns [0, NPRES) hold nodes
    with a winner edge, [NPRES, PADN) nodes without (x = 0). The present
    path needs no validity masking at all, and r/z gates use host-folded
    (Wi_own + Wh) weights; the missing path is Wh-only (3 matmuls).
  - winner cross-memories / e-rows are host-compacted into dense
    feature-major per-core slabs in node order (an edge is last for at
    most one node per direction) and streamed like the own memories:
    one big HWDGE DMA per slab per direction, no per-row transfers.
    (A device-side dma_gather(transpose=True) path from a compacted
    int16-indexed table is kept behind pre_cross=False; it measures
    ~5-10% slower end-to-end due to SWDGE descriptor-generation cost.)
  - time encoding cos(t*bf) enters as a degree-5 polynomial in u=t^2:
    host sends U[6,NPRES] (rows u^m) and vpo = C^T @ WiT_te [6,384].
  - gates: 13 matmuls per 448-node present block (cross/own/e K=128 +
    poly K=6, for r/z/n; Wh_n for the gate-h side). The bh_n bias rides
    a fused (ps_h + bhn) * r scalar_tensor_tensor on DVE.
  - pointwise is software-pipelined in two stages so PSUM banks free
    with minimal PE stalling: stage1 (sigmoids + (ps_h+bhn)*r + ps_n
    add) per block, stage2 (tanh + GRU combine + writeback) deferred
    behind the next block's matmuls. Outputs written feature-major
    bf16; host inverse-permutes/upcasts.
"""
import numpy as np

N_USER = 100000
N_ITEM = 100000
E = 300000
S = 128
T = 128
DE = 128
M = 2 * S + T + DE  # 512

CORES = 8
NPC = 12500
P = 128
BLK = 512            # nodes per compute block (ISA cap: 512 f32 PSUM)
NPBLK = 24           # present blocks
NMBLK = 2            # missing blocks
NPRES = NPBLK * BLK  # 12288
PADN = (NPBLK + NMBLK) * BLK  # 13312
NCT = NPRES + 1      # cross table rows (last row stays zero)
GCH = 896            # gather chunk (SWDGE ring bound: <=~992, %128==0)
DPOLY = 6

_CACHE = {}

OUT_NAMES = ("outuT", "outiT")

_GCHUNKS = []
_c0 = 0
while _c0 < NPRES:
    _n = min(GCH, NPRES - _c0)
    _GCHUNKS.append((_c0, _n))
    _c0 += _n


def _build_program(reps=1, no_gather=False, only_gather=False, gq=4,
                   pre_cross=True, mm_only=False):
    import concourse.bass as bass
    import concourse.mybir as mybir
    import concourse.tile as tile
    from concourse import bacc

    f32 = mybir.dt.float32
    bf16 = mybir.dt.bfloat16
    i16 = mybir.dt.int16

    nc = bacc.Bacc("TRN2", target_bir_lowering=False, debug=False,
                   enable_asserts=True, num_devices=CORES,
                   num_swdge_queues=gq)

    meta = {}
    for d in ("f", "r"):
        meta[d] = dict(
            ownT=nc.dram_tensor(f"ownT_{d}", [P, PADN], bf16, kind="ExternalInput"),
            eT=nc.dram_tensor(f"eT_{d}", [P, NPRES], bf16, kind="ExternalInput"),
            U=nc.dram_tensor(f"U_{d}", [DPOLY, NPRES], bf16, kind="ExternalInput"),
        )
        if pre_cross:
            meta[d]["cT"] = nc.dram_tensor(f"cT_{d}", [P, NPRES], bf16,
                                           kind="ExternalInput")
        else:
            meta[d]["ct"] = nc.dram_tensor(f"ct_{d}", [NCT, S], bf16,
                                           kind="ExternalInput")
            meta[d]["gi"] = nc.dram_tensor(f"gi_{d}", [P, NPRES // 16], i16,
                                           kind="ExternalInput")
    wic_d = nc.dram_tensor("wic", [P, 3 * S], bf16, kind="ExternalInput")
    wie_d = nc.dram_tensor("wie", [P, 3 * S], bf16, kind="ExternalInput")
    wxo_d = nc.dram_tensor("wxo", [P, 3 * S], bf16, kind="ExternalInput")
    whm_d = nc.dram_tensor("whm", [P, 3 * S], bf16, kind="ExternalInput")
    vpo_d = nc.dram_tensor("vpo", [DPOLY, 3 * S], bf16, kind="ExternalInput")
    br_d = nc.dram_tensor("br", [P, 1], f32, kind="ExternalInput")
    bz_d = nc.dram_tensor("bz", [P, 1], f32, kind="ExternalInput")
    bn_d = nc.dram_tensor("bn", [P, 1], f32, kind="ExternalInput")
    bhn_d = nc.dram_tensor("bhn", [P, 1], f32, kind="ExternalInput")

    outu = nc.dram_tensor("outuT", [P, PADN], bf16, kind="ExternalOutput")
    outi = nc.dram_tensor("outiT", [P, PADN], bf16, kind="ExternalOutput")

    AF = mybir.ActivationFunctionType
    OP = mybir.AluOpType

    with tile.TileContext(nc) as tc:
        with tc.tile_pool(name="const", bufs=1) as cpool, \
             tc.tile_pool(name="gat", bufs=1) as gpool, \
             tc.tile_pool(name="str", bufs=2) as spool, \
             tc.tile_pool(name="blk", bufs=2) as bpool, \
             tc.tile_pool(name="ps", bufs=2, space="PSUM") as psum:

            wic = cpool.tile([P, 3 * S], bf16)
            nc.sync.dma_start(wic[:, :], wic_d.ap())
            wie = cpool.tile([P, 3 * S], bf16)
            nc.sync.dma_start(wie[:, :], wie_d.ap())
            wxo = cpool.tile([P, 3 * S], bf16)
            nc.sync.dma_start(wxo[:, :], wxo_d.ap())
            whm = cpool.tile([P, 3 * S], bf16)
            nc.sync.dma_start(whm[:, :], whm_d.ap())
            vpo = cpool.tile([DPOLY, 3 * S], bf16)
            nc.sync.dma_start(vpo[:, :], vpo_d.ap())
            br = cpool.tile([P, 1], f32)
            nc.sync.dma_start(br[:, :], br_d.ap())
            bz = cpool.tile([P, 1], f32)
            nc.sync.dma_start(bz[:, :], bz_d.ap())
            bn = cpool.tile([P, 1], f32)
            nc.sync.dma_start(bn[:, :], bn_d.ap())
            bhn = cpool.tile([P, 1], f32)
            nc.sync.dma_start(bhn[:, :], bhn_d.ap())

            from contextlib import nullcontext
            loop_ctx = tc.For_i(0, reps, 1) if reps > 1 else nullcontext()

            def stage1(ps_r, ps_z, ps_n, ps_h):
                """Release all PSUM banks ASAP: sigmoids + (ps_h+bhn)*r + ps_n."""
                if mm_only:
                    return None
                r = bpool.tile([P, BLK], f32, tag="r", bufs=3)
                nc.scalar.activation(r[:, :], ps_r[:, :], AF.Sigmoid,
                                     bias=br[:, :1])
                z = bpool.tile([P, BLK], bf16, tag="z", bufs=3)
                nc.scalar.activation(z[:, :], ps_z[:, :], AF.Sigmoid,
                                     bias=bz[:, :1])
                t1 = bpool.tile([P, BLK], f32, tag="t1", bufs=3)
                nc.vector.scalar_tensor_tensor(
                    out=t1[:, :], in0=ps_h[:, :], scalar=bhn[:, :1],
                    in1=r[:, :], op0=OP.add, op1=OP.mult)
                if ps_n is not None:
                    nc.vector.tensor_tensor(out=t1[:, :], in0=t1[:, :],
                                            in1=ps_n[:, :], op=OP.add)
                return (z, t1)

            def stage2(st, own, bs, out_d, j0):
                """Deferred tail: tanh + GRU combine + writeback."""
                if st is None:
                    return
                z, t1 = st
                n = bpool.tile([P, BLK], bf16, tag="n", bufs=3)
                nc.scalar.activation(n[:, :], t1[:, :], AF.Tanh,
                                     bias=bn[:, :1])
                dd = bpool.tile([P, BLK], bf16, tag="dd", bufs=3)
                nc.vector.tensor_tensor(out=dd[:, :], in0=own[:, bs],
                                        in1=n[:, :], op=OP.subtract)
                zd = bpool.tile([P, BLK], bf16, tag="zd", bufs=3)
                nc.vector.tensor_tensor(out=zd[:, :], in0=z[:, :],
                                        in1=dd[:, :], op=OP.mult)
                oc = bpool.tile([P, BLK], bf16, tag="oc", bufs=3)
                nc.vector.tensor_tensor(out=oc[:, :], in0=n[:, :],
                                        in1=zd[:, :], op=OP.add)
                nc.scalar.dma_start(out_d.ap()[:, j0:j0 + BLK], oc[:, :])

            UCH = 4 * BLK  # U streamed in 4-block chunks

            with loop_ctx:
              # prefetch: both directions' slabs + idx/table streams up front
              slabs = {}
              for d in ("f", "r"):
                md = meta[d]
                if pre_cross:
                    cgb = gpool.tile([P, NPRES], bf16, tag=f"cgb{d}")
                    nc.scalar.dma_start(cgb[:, :], md["cT"].ap())
                    slabs[d] = dict(cgb=cgb)
                else:
                    gidx = gpool.tile([P, NPRES // 16], i16, tag=f"gidx{d}")
                    nc.scalar.dma_start(gidx[:, :], md["gi"].ap())
                    slabs[d] = dict(gidx=gidx)
                if not only_gather:
                    ownb = gpool.tile([P, PADN], bf16, tag=f"own{d}")
                    nc.sync.dma_start(ownb[:, :], md["ownT"].ap())
                    egb = gpool.tile([P, NPRES], bf16, tag=f"eg{d}")
                    nc.sync.dma_start(egb[:, :], md["eT"].ap())
                    slabs[d]["ownb"] = ownb
                    slabs[d]["egb"] = egb

              for d, out_d in (("f", outi), ("r", outu)):
                md = meta[d]
                if pre_cross:
                    cgb = slabs[d]["cgb"]
                    cg_of = lambda b: cgb[:, b * BLK:(b + 1) * BLK]
                else:
                    gidx = slabs[d]["gidx"]
                    cgs = []
                    for ci, (c0, ncol) in enumerate(_GCHUNKS):
                        cg = gpool.tile([P, 1, GCH], bf16, tag=f"cg{d}{ci}")
                        if not no_gather:
                            nc.gpsimd.dma_gather(
                                cg[:, :, :ncol], md["ct"].ap(),
                                gidx[:, c0 // 16:(c0 + ncol) // 16],
                                num_idxs=ncol, num_idxs_reg=ncol,
                                elem_size=S, transpose=True,
                                queue_num=ci % gq)
                        cgs.append(cg)
                    # gather chunks (896) only tile blocks of 448
                    assert 2 * BLK == GCH, "gather path needs BLK=448"
                    cg_of = lambda b: cgs[b // 2][:, 0,
                                                  (b % 2) * BLK:(b % 2 + 1) * BLK]
                if only_gather:
                    continue
                ownb = slabs[d]["ownb"]
                egb = slabs[d]["egb"]

                # blocks, software-pipelined: stage2(b-1) rides behind
                # stage1(b) so PSUM banks free with minimal PE stalling
                uu = None
                pending = None
                for b in range(NPBLK + NMBLK):
                    j0 = b * BLK
                    present = b < NPBLK
                    if present and b % 4 == 0:
                        uu = spool.tile([DPOLY, UCH], bf16, tag="uu")
                        nc.scalar.dma_start(uu[:, :],
                                            md["U"].ap()[:, j0:j0 + UCH])
                    us = slice((b % 4) * BLK, (b % 4 + 1) * BLK)

                    ps_r = psum.tile([P, BLK], f32, space="PSUM", tag="ps_r")
                    ps_z = psum.tile([P, BLK], f32, space="PSUM", tag="ps_z")
                    ps_h = psum.tile([P, BLK], f32, space="PSUM", tag="ps_h")
                    if present:
                        ps_n = psum.tile([P, BLK], f32, space="PSUM", tag="ps_n")
                        cgv = cg_of(b)
                        for g, ps in enumerate((ps_r, ps_z, ps_n)):
                            gs = slice(g * S, (g + 1) * S)
                            nc.tensor.matmul(ps[:, :], wic[:, gs], cgv,
                                             start=True, stop=False)
                            nc.tensor.matmul(ps[:, :], wxo[:, gs],
                                             ownb[:, j0:j0 + BLK],
                                             start=False, stop=False)
                            nc.tensor.matmul(ps[:, :], wie[:, gs],
                                             egb[:, j0:j0 + BLK],
                                             start=False, stop=False)
                            nc.tensor.matmul(ps[:, :], vpo[:, gs], uu[:, us],
                                             start=False, stop=True)
                        nc.tensor.matmul(ps_h[:, :], whm[:, 2 * S:3 * S],
                                         ownb[:, j0:j0 + BLK],
                                         start=True, stop=True)
                    else:
                        ps_n = None
                        for g, ps in enumerate((ps_r, ps_z, ps_h)):
                            gs = slice(g * S, (g + 1) * S)
                            nc.tensor.matmul(ps[:, :], whm[:, gs],
                                             ownb[:, j0:j0 + BLK],
                                             start=True, stop=True)
                    st = stage1(ps_r, ps_z, ps_n, ps_h)
                    if pending is not None:
                        stage2(*pending)
                    pending = (st, ownb, slice(j0, j0 + BLK), out_d, j0)
                if pending is not None:
                    stage2(*pending)

    nc.compile()
    return nc


def _np_gru(x, h, Wi, Wh, bi, bh):
    gi = x @ Wi.T + bi
    gh = h @ Wh.T + bh
    gir, giz, gin = np.split(gi, 3, axis=1)
    ghr, ghz, ghn = np.split(gh, 3, axis=1)
    r = 1.0 / (1.0 + np.exp(-(gir + ghr)))
    z = 1.0 / (1.0 + np.exp(-(giz + ghz)))
    n = np.tanh(gin + r * ghn)
    return (1.0 - z) * n + z * h


def _host_prep(si, sj, t, e, src, dst, Wi, Wh, bi, bh, basis_freq):
    import ml_dtypes
    bf16 = ml_dtypes.bfloat16

    eid = np.arange(E, dtype=np.int64)
    lastf = np.full(N_ITEM, -1, dtype=np.int64)
    lastf[dst.astype(np.int64)] = eid
    lastr = np.full(N_USER, -1, dtype=np.int64)
    lastr[src.astype(np.int64)] = eid

    # te poly: cos(t*f) = sum_m C[k,m] * (t^2)^m,  C[k,m] = (-1)^m f^(2m)/(2m)!
    import math
    bf = np.asarray(basis_freq, np.float64)
    fact = np.array([math.factorial(2 * m) for m in range(DPOLY)], np.float64)
    C = np.stack([((-1.0) ** m) * bf ** (2 * m) / fact[m]
                  for m in range(DPOLY)], axis=1)  # [T, DPOLY]
    WiT = np.ascontiguousarray(Wi.T).astype(np.float64)
    WhT = np.ascontiguousarray(Wh.T).astype(np.float64)
    vpo = (C.T @ WiT[2 * S:2 * S + T]).astype(np.float32)  # [DPOLY, 384]

    wic = WiT[0:S].astype(bf16)
    wie = WiT[2 * S + T:].astype(bf16)
    wxo = WiT[S:2 * S].copy()
    wxo[:, :2 * S] += WhT[:, :2 * S]  # fold Wh into Wi_own for r,z gates
    wxo = wxo.astype(bf16)
    whm = WhT.astype(bf16)
    br = (bi[:S] + bh[:S]).reshape(P, 1).astype(np.float32)
    bz = (bi[S:2 * S] + bh[S:2 * S]).reshape(P, 1).astype(np.float32)
    bn = bi[2 * S:].reshape(P, 1).astype(np.float32)
    bhn = bh[2 * S:].reshape(P, 1).astype(np.float32)

    t64 = np.asarray(t, np.float64)
    patches = []   # (side 'u'/'i', node_global_ids, x_rows, h_rows)
    in_maps = []
    colmaps = []

    def meta_for(w, own_rows, cross_idx_all, cross_mem, t64, e):
        npc = len(w)
        present = np.nonzero(w >= 0)[0]
        missing = np.nonzero(w < 0)[0]
        over_p = present[NPRES:] if len(present) > NPRES else np.empty(0, np.int64)
        over_m = missing[NMBLK * BLK:] if len(missing) > NMBLK * BLK else np.empty(0, np.int64)
        present = present[:NPRES]
        missing = missing[:NMBLK * BLK]
        np_, nm_ = len(present), len(missing)

        col_of = np.full(npc, PADN - 1, np.int64)
        col_of[present] = np.arange(np_)
        col_of[missing] = NPRES + np.arange(nm_)

        wp = w[present]
        u = t64[wp] ** 2
        U = np.zeros((DPOLY, NPRES), np.float32)
        for m in range(DPOLY):
            U[m, :np_] = u ** m
        eT = np.zeros((P, NPRES), np.float32)
        eT[:, :np_] = e[wp].T
        ownT = np.zeros((P, PADN), np.float32)
        ownT[:, :np_] = own_rows[present].T
        ownT[:, NPRES:NPRES + nm_] = own_rows[missing].T

        cids = cross_idx_all[wp].astype(np.int64)
        uniq, inv = np.unique(cids, return_inverse=True)
        ct = np.zeros((NCT, S), np.float32)
        ct[:len(uniq)] = cross_mem[uniq]
        idx = np.full(NPRES, NCT - 1, np.int32)
        idx[:np_] = inv
        idx16 = idx.astype(np.int16)
        gi = np.tile(idx16.reshape(-1, 16).T, (8, 1)).copy()
        cT = np.zeros((P, NPRES), np.float32)
        cT[:, :np_] = cross_mem[cids].T

        m = dict(ct=ct.astype(bf16), ownT=ownT.astype(bf16),
                 eT=eT.astype(bf16), U=U.astype(bf16), gi=gi,
                 cT=cT.astype(bf16))
        return m, col_of, over_p, over_m, wp

    for c in range(CORES):
        sl = slice(c * NPC, (c + 1) * NPC)
        im = {"wic": wic, "wie": wie, "wxo": wxo, "whm": whm,
              "vpo": vpo.astype(bf16), "br": br, "bz": bz, "bn": bn,
              "bhn": bhn}
        cm = {}
        for d, w_all, own_rows, cross_idx_all, cross_mem, side in (
                ("f", lastf[sl], sj[sl], src, si, "i"),
                ("r", lastr[sl], si[sl], dst, sj, "u")):
            m, col_of, over_p, over_m, wp = meta_for(
                w_all, own_rows, cross_idx_all, cross_mem, t64, e)
            for k, v in m.items():
                im[f"{k}_{d}"] = v
            cm[d] = col_of
            for over, is_present in ((over_p, True), (over_m, False)):
                if len(over):
                    h_rows = own_rows[over]
                    if is_present:
                        wo = w_all[over]
                        te = np.cos(np.asarray(t, np.float32)[wo][:, None]
                                    * np.asarray(basis_freq, np.float32)[None, :])
                        x = np.concatenate(
                            [cross_mem[cross_idx_all[wo].astype(np.int64)],
                             h_rows, te, e[wo]], axis=1)
                    else:
                        x = np.zeros((len(over), M), np.float32)
                    out = _np_gru(x, h_rows, Wi, Wh, bi, bh)
                    patches.append((c, side, over, out))
        in_maps.append(im)
        colmaps.append(cm)
    return in_maps, colmaps, patches


def _postprocess(results, colmaps, patches):
    si_new = np.empty((N_USER, S), np.float32)
    sj_new = np.empty((N_ITEM, S), np.float32)
    for c in range(CORES):
        sl = slice(c * NPC, (c + 1) * NPC)
        outu = np.asarray(results[c]["outuT"]).T.astype(np.float32)
        outi = np.asarray(results[c]["outiT"]).T.astype(np.float32)
        si_new[sl] = outu[colmaps[c]["r"]]
        sj_new[sl] = outi[colmaps[c]["f"]]
    for c, side, nodes, out in patches:
        if side == "u":
            si_new[c * NPC + nodes] = out
        else:
            sj_new[c * NPC + nodes] = out
    return si_new, sj_new


def kernel(si, sj, t, e, src, dst, Wi, Wh, bi, bh, basis_freq):
    from concourse import bass_utils

    si = np.asarray(si, np.float32)
    sj = np.asarray(sj, np.float32)
    t = np.asarray(t, np.float32)
    e = np.asarray(e, np.float32)
    src = np.asarray(src, np.int32)
    dst = np.asarray(dst, np.int32)
    Wi = np.asarray(Wi, np.float32)
    Wh = np.asarray(Wh, np.float32)
    bi = np.asarray(bi, np.float32)
    bh = np.asarray(bh, np.float32)
    basis_freq = np.asarray(basis_freq, np.float32)

    if "nc" not in _CACHE:
        _CACHE["nc"] = _build_program()
    nc = _CACHE["nc"]

    in_maps, colmaps, patches = _host_prep(si, sj, t, e, src, dst,
                                           Wi, Wh, bi, bh, basis_freq)
    res = bass_utils.run_bass_kernel_spmd(nc, in_maps, core_ids=list(range(CORES)))
    return _postprocess(res.results, colmaps, patches)


import concourse.bass as bass  # noqa: E402
import concourse.mybir as mybir  # noqa: E402
